# revision 1
# baseline (speedup 1.0000x reference)
"""Trainium2 Bass kernel for gathered-row MLP decode matmul.

out[b, 0, r] = sum_d x[b, 0, d] * weight[indices[r], d]

Active path (kernel() -> run_full(mode="hi") -> _build_hi): dedup+sort the
indices on the host and shard them across 8 cores (~452 rows each). The
weight is cast to fp16 on the host (one 2-byte copy instead of an fp32 or
hi/lo pair: halves both HBM traffic and PE work; end-to-end scale-rel
error ~2.4e-4 against the 2e-2 gate). Each core gathers its rows in
128-row transpose-gather chunks that land matmul-ready [d%128, k, r];
the %128 tail chunk uses a non-transpose gather plus PE transposes that
hide under the other chunks' DMA transfers. Matmuls are weight-stationary
(lhsT = gathered [128, rows] tile, moving = x^T fp16 [128, 32] per
contraction tile), accumulating out^T [rows, 32] in PSUM; each chunk is
copied to SBUF and DMAed out transposed. The host transposes/assembles
per-core outputs and inverse-maps duplicates back to the original 4403
index order. The older fp32-accurate hi/lo modes are kept in _build()
for reference/fallback.
"""
import os
import sys
from contextlib import ExitStack

sys.path.insert(0, "/opt/trn_rl_repo")
os.environ.setdefault("MYCRO_LOCAL_CACHE", "1")

import numpy as np

D_FF = 11008
D_MODEL = 4096
R_TOTAL = 4403
B = 32
NCORES = 8
P = 128
KT = D_MODEL // P          # 32 contraction tiles
NPAD = 640                 # padded per-core index count (5*128), fallback
NPAD_DEDUP = 512           # padded per-core count for the dedup path
LO_SCALE = 2048.0          # wlo/xlo pre-scale (2^11)

# per-core share of the real 4403 indices (no-dedup fallback)
_CORE_N = [551, 551, 551, 550, 550, 550, 550, 550]
_CORE_START = [0]
for _n in _CORE_N[:-1]:
    _CORE_START.append(_CORE_START[-1] + _n)

_cache = {}


def _build(
    reps=1, mode="full", tiny_out=False, npad=NPAD, chunks=None, gbufs=2,
    nvalid=None,
):
    """mode: full (3-matmul) | fused (2-pass M=64 packing) | dma (gathers
    only) | dma_nt (non-transpose gathers) | mm (matmuls only).
    tiny_out: shrink the DRAM output to [B, 64] so bench-loop host
    transfers are negligible (timing only)."""
    if nvalid is None:
        nvalid = npad
    key = ("nc", reps, mode, tiny_out, npad, chunks, gbufs, nvalid)
    if key in _cache:
        return _cache[key]
    from concourse import bacc, mybir, tile

    f32 = mybir.dt.float32
    f16 = mybir.dt.float16
    i16 = mybir.dt.int16

    if chunks is None:
        chunks = tuple((i, min(256, npad - i)) for i in range(0, npad, 256))

    nc = bacc.Bacc(
        "TRN2", target_bir_lowering=False, debug=False, enable_asserts=False
    )
    whi_dram = nc.dram_tensor("whi", [D_FF, D_MODEL], f16, kind="ExternalInput").ap()
    wlo_dram = nc.dram_tensor("wlo", [D_FF, D_MODEL], f16, kind="ExternalInput").ap()
    if mode in ("fused", "fused2"):
        xp_dram = nc.dram_tensor("xp", [P, KT * 2 * B], f16, kind="ExternalInput").ap()
        if mode == "fused2":
            combm_dram = nc.dram_tensor(
                "combm", [P, B], f32, kind="ExternalInput"
            ).ap()
    else:
        xh_dram = nc.dram_tensor("xh", [P, KT * B], f16, kind="ExternalInput").ap()
        xl_dram = nc.dram_tensor("xl", [P, KT * B], f16, kind="ExternalInput").ap()
    idx_dram = nc.dram_tensor("idx", [P, npad // 16], i16, kind="ExternalInput").ap()
    out_cols = 64 if tiny_out else npad
    out_dram = nc.dram_tensor("out", [B, out_cols], f32, kind="ExternalOutput").ap()

    with tile.TileContext(nc) as tc, ExitStack() as ctx:
        consts = ctx.enter_context(tc.tile_pool(name="consts", bufs=1))
        whi_pool = ctx.enter_context(tc.tile_pool(name="whiT", bufs=gbufs))
        wlo_pool = ctx.enter_context(tc.tile_pool(name="wloT", bufs=gbufs))
        psum = ctx.enter_context(tc.tile_pool(name="psum", bufs=4, space="PSUM"))
        out_pool = ctx.enter_context(tc.tile_pool(name="outp", bufs=2))

        # idx first: the gathers (the critical path) depend only on it
        idx_sb = consts.tile([P, npad // 16], i16)
        nc.sync.dma_start(idx_sb[:], idx_dram)
        if mode in ("fused", "fused2"):
            xp_sb = consts.tile([P, KT * 2 * B], f16)
            nc.sync.dma_start(xp_sb[:], xp_dram)
        else:
            xh_sb = consts.tile([P, KT * B], f16)
            nc.sync.dma_start(xh_sb[:], xh_dram)
            xl_sb = consts.tile([P, KT * B], f16)
            nc.sync.dma_start(xl_sb[:], xl_dram)

        if mode == "mm":
            whiT_c = consts.tile([P, KT, 256], f16)
            nc.gpsimd.memset(whiT_c[:], 0.25)
            wloT_c = consts.tile([P, KT, 256], f16)
            nc.gpsimd.memset(wloT_c[:], 0.25)

        if mode == "fused2":
            # combM.T @ [hi; lo; 0] = hi + lo/2048 (host-built constant;
            # K padded to 128 so the fp32 matmul uses the proven full
            # partition-group shape)
            combM = consts.tile([P, B], f32)
            nc.sync.dma_start(combM[:], combm_dram)

        for _rep in range(reps):
            out_sb = out_pool.tile([B, npad], f32, tag="out_sb")
            if mode == "fused":
                t1_sb = out_pool.tile([64, npad], f32, tag="t1")
                outA_sb = out_pool.tile([B, npad], f32, tag="outA")

            for c, (r0, ncols) in enumerate(chunks):
                if mode in ("full", "fused", "fused2", "dma"):
                    # valid (non -1) indices in this chunk; the SWDGE trims
                    # the transfer to the valid prefix, so -1 tail padding
                    # costs no DMA.
                    nval_c = max(0, min(nvalid - r0, ncols))
                    # whiT[p, k, i] = whi[idx[r0+i], k*128 + p]
                    whiT = whi_pool.tile([P, KT, ncols], f16, tag="whiT")
                    wloT = wlo_pool.tile([P, KT, ncols], f16, tag="wloT")
                    if _rep == 0 and nval_c < ncols:
                        # first use of the slot: zero the never-gathered tail
                        # columns so downstream matmuls read finite data
                        nc.vector.memset(whiT[:, :, nval_c:], 0)
                        nc.vector.memset(wloT[:, :, nval_c:], 0)
                    nc.gpsimd.dma_gather(
                        out_ap=whiT[:],
                        in_ap=whi_dram,
                        idxs_ap=idx_sb[:, r0 // 16 : (r0 + ncols) // 16],
                        num_idxs=ncols,
                        num_idxs_reg=nval_c,
                        elem_size=D_MODEL,
                        transpose=True,
                    )
                    nc.gpsimd.dma_gather(
                        out_ap=wloT[:],
                        in_ap=wlo_dram,
                        idxs_ap=idx_sb[:, r0 // 16 : (r0 + ncols) // 16],
                        num_idxs=ncols,
                        num_idxs_reg=nval_c,
                        elem_size=D_MODEL,
                        transpose=True,
                    )
                elif mode == "dma_nt":
                    whiT = whi_pool.tile([P, -(-ncols // P), D_MODEL], f16, tag="whiT")
                    nc.gpsimd.dma_gather(
                        out_ap=whiT[:],
                        in_ap=whi_dram,
                        idxs_ap=idx_sb[:, r0 // 16 : (r0 + ncols) // 16],
                        num_idxs=ncols,
                        num_idxs_reg=ncols,
                        elem_size=D_MODEL,
                        transpose=False,
                    )
                    wloT = wlo_pool.tile([P, -(-ncols // P), D_MODEL], f16, tag="wloT")
                    nc.gpsimd.dma_gather(
                        out_ap=wloT[:],
                        in_ap=wlo_dram,
                        idxs_ap=idx_sb[:, r0 // 16 : (r0 + ncols) // 16],
                        num_idxs=ncols,
                        num_idxs_reg=ncols,
                        elem_size=D_MODEL,
                        transpose=False,
                    )
                else:
                    whiT = whiT_c
                    wloT = wloT_c

                if mode in ("dma", "dma_nt"):
                    continue

                if mode in ("fused", "fused2"):
                    # One PSUM chain: rows 0-31 accumulate xh*whi (hi chain);
                    # rows 32-63 accumulate xl_s*whi (mm1) AND xh*wlo_s (mm2).
                    # The group is opened by mm1@k=0 (spans rows 0-63) and
                    # closed by mm1@k=31, so mm2@k=31 is emitted before it.
                    psAB = psum.tile([64, ncols], mybir.dt.float32, tag="psA")

                    def mm1(k):
                        nc.tensor.matmul(
                            out=psAB[:],
                            lhsT=xp_sb[:, k * 2 * B : (k + 1) * 2 * B],
                            rhs=whiT[:, k, :],
                            start=(k == 0),
                            stop=(k == KT - 1),
                        )

                    def mm2(k):
                        nc.tensor.matmul(
                            out=psAB[B : 2 * B, :],
                            lhsT=xp_sb[:, k * 2 * B : k * 2 * B + B],
                            rhs=wloT[:, k, :],
                            start=False,
                            stop=False,
                        )

                    # mm1s first: they only depend on the whi gather, so the
                    # PE starts before wlo lands. mm1@KT-1 closes the group.
                    for k in range(KT - 1):
                        mm1(k)
                    for k in range(KT):
                        mm2(k)
                    mm1(KT - 1)
                    if mode == "fused2":
                        # recombine on the PE: out = combM.T @ [hi; lo; 0]
                        # = hi + lo/2048, landing directly on partitions 0-31
                        cmb_sb = out_pool.tile([P, ncols], f32, tag="cmb")
                        if _rep == 0 and c < 2:
                            nc.vector.memset(cmb_sb[2 * B :, :], 0)
                        nc.vector.tensor_copy(cmb_sb[: 2 * B, :], psAB[:])
                        psO = psum.tile([B, ncols], mybir.dt.float32, tag="psO")
                        nc.tensor.matmul(
                            out=psO[:], lhsT=combM[:], rhs=cmb_sb[:],
                            start=True, stop=True,
                        )
                        nc.scalar.copy(out_sb[:, r0 : r0 + ncols], psO[:])
                        if not tiny_out:
                            nc.sync.dma_start(
                                out_dram[:, r0 : r0 + ncols],
                                out_sb[:, r0 : r0 + ncols],
                            )
                        continue
                    # hi chain copied to partitions 0-31; lo-sum scaled on
                    # 32-63, realigned to 0-31 with an HWDGE SBUF-SBUF DMA
                    # (keeps the tail off the SWDGE queue the gathers use),
                    # then added and written out per chunk.
                    nc.scalar.copy(outA_sb[:, r0 : r0 + ncols], psAB[:B, :])
                    nc.vector.tensor_scalar_mul(
                        t1_sb[B : 2 * B, r0 : r0 + ncols],
                        psAB[B : 2 * B, :],
                        1.0 / LO_SCALE,
                    )
                    nc.sync.dma_start(
                        out_sb[:, r0 : r0 + ncols],
                        t1_sb[B : 2 * B, r0 : r0 + ncols],
                    )
                    nc.vector.tensor_add(
                        out_sb[:, r0 : r0 + ncols],
                        out_sb[:, r0 : r0 + ncols],
                        outA_sb[:, r0 : r0 + ncols],
                    )
                    if not tiny_out:
                        nc.sync.dma_start(
                            out_dram[:, r0 : r0 + ncols],
                            out_sb[:, r0 : r0 + ncols],
                        )
                    continue

                # mode full/mm: 3 matmul passes, both chains on partitions 0-31
                mcols = 256 if mode == "mm" else ncols
                psA = psum.tile([B, mcols], mybir.dt.float32, tag="psA")
                psB = psum.tile([B, mcols], mybir.dt.float32, tag="psB")
                for k in range(KT):
                    xh_k = xh_sb[:, k * B : (k + 1) * B]
                    xl_k = xl_sb[:, k * B : (k + 1) * B]
                    nc.tensor.matmul(
                        out=psA[:],
                        lhsT=xh_k,
                        rhs=whiT[:, k, :mcols],
                        start=(k == 0),
                        stop=(k == KT - 1),
                    )
                    nc.tensor.matmul(
                        out=psB[:],
                        lhsT=xh_k,
                        rhs=wloT[:, k, :mcols],
                        start=(k == 0),
                        stop=False,
                    )
                    nc.tensor.matmul(
                        out=psB[:],
                        lhsT=xl_k,
                        rhs=whiT[:, k, :mcols],
                        start=False,
                        stop=(k == KT - 1),
                    )
                dst = out_sb[:, r0 : r0 + ncols]
                nc.scalar.mul(dst, psB[:, :ncols], 1.0 / LO_SCALE)
                nc.vector.tensor_add(dst, dst, psA[:, :ncols])

            if mode == "fused":
                if tiny_out:
                    nc.sync.dma_start(out_dram, outA_sb[:, :out_cols])
                continue
            if mode == "fused2":
                continue
            if mode in ("dma", "dma_nt"):
                nc.vector.tensor_copy(out_sb[:, :64], whiT[:32, 0, :64])
            nc.sync.dma_start(out_dram, out_sb[:, :out_cols])

    nc.compile()
    _cache[key] = nc
    return nc


def _prep_scatter(nc, out_dram, obs, idx_sb, nv16, c):
    B_ = B
    nc.gpsimd.dma_scatter_add(
        out_ap=out_dram,
        in_ap=obs[:, c : c + 1, :],
        idxs_ap=idx_sb[:, nv16 + c * 8 : nv16 + c * 8 + 8],
        num_idxs=128,
        num_idxs_reg=128,
        elem_size=2 * B_,
        prepare_only=True,
        queue_num=1,
    )


def _build_hi(nvalid, gbufs=3, reps=1):
    """fp16-only weight-stationary kernel.

    One fp16 gather per 128-row chunk lands matmul-ready [d%128, k, r].
    Matmuls are weight-stationary: lhsT = gathered chunk [128, ncols<=128],
    rhs = x^T fp16 [128, 32] per contraction tile, accumulating
    psT[r, b] over the 32 k-tiles. Output is written transposed
    [nvalid, B]; the host transposes during assembly. fp16 on both sides
    gives ~2.4e-4 scale-rel error (gate is 2e-2).
    """
    key = ("hi5", nvalid, gbufs, reps)
    if key in _cache:
        return _cache[key]
    from concourse import bacc, mybir, tile

    f32 = mybir.dt.float32
    f16 = mybir.dt.float16
    i16 = mybir.dt.int16

    nfull = nvalid // 128      # full 128-row transpose-gather chunks
    nt = nvalid % 128          # odd-size tail chunk, non-transpose gather
    nch = nfull + (1 if nt else 0)
    nv16 = nvalid // 16

    nc = bacc.Bacc(
        "TRN2",
        target_bir_lowering=False,
        debug=False,
        enable_asserts=False,
        num_swdge_queues=2,
    )
    whi_dram = nc.dram_tensor("whi", [D_FF, D_MODEL], f16, kind="ExternalInput").ap()
    xh_dram = nc.dram_tensor("xh", [P, KT * B], f16, kind="ExternalInput").ap()
    if nt:
        id_dram = nc.dram_tensor("ident", [nt, nt], f16, kind="ExternalInput").ap()
    # first nv16 cols: gather indices; next nv16: iota rows for the scatter
    idx_dram = nc.dram_tensor("idx", [P, 2 * nv16], i16, kind="ExternalInput").ap()
    # 64-wide rows so the scatter elem is 256B (cols 32..63 are zero pad)
    out_dram = nc.dram_tensor("out", [nvalid, 2 * B], f32, kind="ExternalOutput").ap()

    with tile.TileContext(nc) as tc, ExitStack() as ctx:
        consts = ctx.enter_context(tc.tile_pool(name="consts", bufs=1))
        whi_pool = ctx.enter_context(tc.tile_pool(name="whiT", bufs=max(gbufs, nch)))
        psum = ctx.enter_context(tc.tile_pool(name="psum", bufs=4, space="PSUM"))

        idx_sb = consts.tile([P, 2 * nv16], i16)
        nc.sync.dma_start(idx_sb[:], idx_dram)
        xh_sb = consts.tile([P, KT * B], f16)
        nc.sync.dma_start(xh_sb[:], xh_dram)
        obs = consts.tile([P, nch, 2 * B], f32)
        nc.vector.memset(obs[:], 0)
        if nt:
            id_sb = consts.tile([nt, nt], f16)
            nc.sync.dma_start(id_sb[:], id_dram)
            # partitions >= nt are never written or read: no memset needed
            w4 = consts.tile([P, 1, D_MODEL], f16)

        # issue all gathers + output-scatter preps first so the Pool SEQ is
        # never blocked behind a trigger's data wait; desc-gen pipelines
        # ahead of the serialized DMA transfers
        whiTs = []
        if nt:
            # odd tail gathered FIRST (its transfer is the shortest and its
            # PE transposes hide under the remaining gathers' transfers)
            nc.gpsimd.dma_gather(
                out_ap=w4[:],
                in_ap=whi_dram,
                idxs_ap=idx_sb[:, nfull * 8 : nv16],
                num_idxs=nt,
                num_idxs_reg=nt,
                elem_size=D_MODEL,
                transpose=False,
            )
        for c in range(nfull):
            r0 = c * 128
            whiT = whi_pool.tile([P, KT, 128], f16, tag=f"whiT{c}")
            if c == nfull - 1:
                # last chunk in two half-row gathers: the k<16 matmuls can
                # start while the second half is still in flight
                for h in range(4):
                    nc.gpsimd.dma_gather(
                        out_ap=whiT[:, h * 8 : (h + 1) * 8, :],
                        in_ap=whi_dram[:, h * 1024 : (h + 1) * 1024],
                        idxs_ap=idx_sb[:, r0 // 16 : r0 // 16 + 8],
                        num_idxs=128,
                        num_idxs_reg=128,
                        elem_size=1024,
                        elem_step=D_MODEL,
                        transpose=True,
                    )
            else:
                nc.gpsimd.dma_gather(
                    out_ap=whiT[:],
                    in_ap=whi_dram,
                    idxs_ap=idx_sb[:, r0 // 16 : r0 // 16 + 8],
                    num_idxs=128,
                    num_idxs_reg=128,
                    elem_size=D_MODEL,
                    transpose=True,
                )
            whiTs.append(whiT)
        if nt:
            # on-chip transpose of the tail rows into matmul-ready layout
            whiT_nt = whi_pool.tile([P, KT, nt], f16, tag="whiTnt")
            for k in range(KT):
                psX = psum.tile([P, nt], f16, tag="psX")
                nc.tensor.transpose(
                    psX[:], w4[:nt, 0, k * P : (k + 1) * P], id_sb[:]
                )
                nc.vector.tensor_copy(whiT_nt[:, k, :], psX[:])
            whiTs.append(whiT_nt)

        order = list(range(nch))
        if nt and nch >= 2:
            # whiTs[nch-1] is the nt chunk (data ready early); run it before
            # the last full chunk, whose gather finishes last
            order = order[: nch - 2] + [nch - 1, nch - 2]
        for c in order:
            whiT = whiTs[c]
            rows = nt if (nt and c == nch - 1) else 128
            r0 = nfull * 128 if (nt and c == nch - 1) else c * 128
            psT = psum.tile([rows, B], f32, tag="psT")
            for k in range(KT):
                nc.tensor.matmul(
                    out=psT[:],
                    lhsT=whiT[:, k, :],
                    rhs=xh_sb[:, k * B : (k + 1) * B],
                    start=(k == 0),
                    stop=(k == KT - 1),
                )
            nc.scalar.copy(obs[:rows, c, :B], psT[:])
            nc.sync.dma_start(out_dram[r0 : r0 + rows, :B], obs[:rows, c, :B])

    nc.compile()
    _cache[key] = nc
    return nc


def _make_in_maps_hi(x, weight, indices):
    """Host prep for the hi kernel: dedup+shard indices, fp16 casts.

    Returns (in_maps, assemble_fn, nvalid)."""
    x = np.asarray(x, dtype=np.float32)
    weight = np.asarray(weight, dtype=np.float32)
    indices = np.asarray(indices, dtype=np.int64)

    whi = np.ascontiguousarray(weight.astype(np.float16))
    xt = np.ascontiguousarray(
        x[:, 0, :].reshape(B, KT, P).transpose(2, 1, 0).reshape(P, KT * B)
    )
    xh = np.ascontiguousarray(xt.astype(np.float16))

    uniq, inv = np.unique(indices, return_inverse=True)
    nu = len(uniq)
    base, rem = divmod(nu, NCORES)
    counts = [base + (1 if c < rem else 0) for c in range(NCORES)]
    starts = np.concatenate([[0], np.cumsum(counts)[:-1]])
    # multiples of 128 go through transpose gathers; the %16 tail is a
    # non-transpose gather + on-chip PE transpose
    nvalid = -(-max(counts) // 16) * 16

    iota = _wrap_idx(np.arange(nvalid, dtype=np.int16))
    ident = np.eye(nvalid % 128 or 1, dtype=np.float16)
    in_maps = []
    for c in range(NCORES):
        idx_pad = np.zeros(nvalid, dtype=np.int16)
        idx_pad[: counts[c]] = uniq[starts[c] : starts[c] + counts[c]]
        idx_full = np.concatenate([_wrap_idx(idx_pad), iota], axis=1)
        in_maps.append(
            {
                "whi": whi,
                "xh": xh,
                "idx": np.ascontiguousarray(idx_full),
                "ident": ident,
            }
        )

    def assemble(results):
        cols = np.empty((B, nu), dtype=np.float32)
        for c in range(NCORES):
            cols[:, starts[c] : starts[c] + counts[c]] = (
                results[c]["out"][: counts[c], :B].T
            )
        return np.ascontiguousarray(cols[:, inv].reshape(B, 1, R_TOTAL))

    return in_maps, assemble, nvalid


def _split_pair(a):
    """fp32 -> (hi fp16, (a-hi)*2048 fp16). hi + lo/2048 ~= a to ~2^-22 rel."""
    hi = a.astype(np.float16)
    lo = ((a - hi.astype(np.float32)) * LO_SCALE).astype(np.float16)
    return hi, lo


def _wrap_idx(idx_pad):
    """[npad] int16 -> [128, npad//16] wrapped-16 layout, replicated 8x."""
    npad = idx_pad.shape[0]
    blk = idx_pad.reshape(npad // 16, 16).T  # [16, npad//16]
    return np.ascontiguousarray(np.tile(blk, (8, 1)))


def _make_in_maps(x, weight, indices, dedup=True):
    """Returns (in_maps, assemble_fn, npad)."""
    x = np.asarray(x, dtype=np.float32)
    weight = np.ascontiguousarray(np.asarray(weight, dtype=np.float32))
    indices = np.asarray(indices, dtype=np.int64)

    whi, wlo = _split_pair(weight)
    whi = np.ascontiguousarray(whi)
    wlo = np.ascontiguousarray(wlo)

    # x^T staged so the DMA is contiguous: xt[p, k*32+b] = x[b, 0, k*128+p]
    xt = np.ascontiguousarray(
        x[:, 0, :].reshape(B, KT, P).transpose(2, 1, 0).reshape(P, KT * B)
    )
    xh, xl = _split_pair(xt)
    # packed [xh || xl] per contraction tile for the fused M=64 matmul
    xp = np.empty((P, KT, 2 * B), dtype=np.float16)
    xp[:, :, :B] = xh.reshape(P, KT, B)
    xp[:, :, B:] = xl.reshape(P, KT, B)
    xp = np.ascontiguousarray(xp.reshape(P, KT * 2 * B))
    combm = np.zeros((P, B), dtype=np.float32)
    combm[:B, :] = np.eye(B, dtype=np.float32)
    combm[B : 2 * B, :] = np.eye(B, dtype=np.float32) / LO_SCALE

    uniq, inv = np.unique(indices, return_inverse=True)
    nu = len(uniq)
    use_dedup = dedup and -(-nu // NCORES) <= NPAD_DEDUP
    if use_dedup:
        npad = NPAD_DEDUP
        base, rem = divmod(nu, NCORES)
        counts = [base + (1 if c < rem else 0) for c in range(NCORES)]
        starts = np.concatenate([[0], np.cumsum(counts)[:-1]])
        core_idx = [uniq[starts[c] : starts[c] + counts[c]] for c in range(NCORES)]
    else:
        npad = NPAD
        counts = list(_CORE_N)
        starts = list(_CORE_START)
        core_idx = [
            indices[starts[c] : starts[c] + counts[c]] for c in range(NCORES)
        ]

    # uniform valid count (dup-padded with row 0); -1 beyond it is trimmed
    # from the gather transfer by the SWDGE
    nvalid = min(-(-max(counts) // 16) * 16, npad)

    in_maps = []
    for c in range(NCORES):
        idx_pad = np.full(npad, -1, dtype=np.int16)
        idx_pad[: counts[c]] = core_idx[c]
        idx_pad[counts[c] : nvalid] = 0
        in_maps.append(
            {
                "whi": whi,
                "wlo": wlo,
                "xh": np.ascontiguousarray(xh),
                "xl": np.ascontiguousarray(xl),
                "xp": xp,
                "combm": combm,
                "idx": _wrap_idx(idx_pad),
            }
        )

    def assemble(results):
        cols = np.empty((B, sum(counts)), dtype=np.float32)
        for c in range(NCORES):
            cols[:, starts[c] : starts[c] + counts[c]] = results[c]["out"][
                :, : counts[c]
            ]
        if use_dedup:
            out = cols[:, inv]
        else:
            out = cols
        return np.ascontiguousarray(out.reshape(B, 1, R_TOTAL))

    return in_maps, assemble, npad, nvalid


def _filter_in_maps(nc, in_maps):
    names = set()
    from concourse import mybir

    for alloc in nc.m.functions[0].allocations:
        if isinstance(alloc, mybir.MemoryLocationSet) and alloc.kind == "ExternalInput":
            names.add(alloc.memorylocations[0].name)
    return [{k: v for k, v in m.items() if k in names} for m in in_maps]


def run_full(x, weight, indices, trace=False, mode="hi", dedup=True):
    """Returns (output, BassKernelResults)."""
    from concourse.bass_utils import run_bass_kernel_spmd

    if mode == "hi":
        in_maps, assemble, nvalid = _make_in_maps_hi(x, weight, indices)
        nc = _build_hi(nvalid)
        in_maps = _filter_in_maps(nc, in_maps)
        res = run_bass_kernel_spmd(nc, in_maps, list(range(NCORES)), trace=trace)
        return assemble(res.results), res

    in_maps, assemble, npad, nvalid = _make_in_maps(x, weight, indices, dedup=dedup)
    if mode == "fused2" and npad != NPAD_DEDUP:
        # the fused2 epilogue is only validated for the 512-wide dedup
        # layout; the rare >4096-unique fallback uses the fused tail
        mode = "fused"
    nc = _build(1, mode, False, npad, nvalid=nvalid)
    in_maps = _filter_in_maps(nc, in_maps)
    res = run_bass_kernel_spmd(nc, in_maps, list(range(NCORES)), trace=trace)
    return assemble(res.results), res


def kernel(x, weight, indices):
    out, _ = run_full(x, weight, indices)
    return out



# revision 35
# speedup vs baseline: 1.5008x; 1.5008x over previous
"""Trainium2 Bass kernel for gathered-row MLP decode matmul.

out[b, 0, r] = sum_d x[b, 0, d] * weight[indices[r], d]

Active path (kernel() -> run_full(mode="hi8") -> _build_hi8): dedup the
indices on the host, shard the ~3629 unique rows across 8 cores (~454
each, padded to 512 = 4x128 with duplicate indices). The weight is cast
to fp8 e3m4 on the host with a x128 prescale (4-bit mantissa; end-to-end
scale-rel error ~1.2e-2 against the 2e-2 gate), halving HBM gather
traffic vs fp16. Each core issues 128-row transpose gathers (the last
chunk as two half-column gathers so its matmuls start earlier). The
transpose DMA works at 16-bit granularity, so fp8 values land as
interleaved even/odd pairs [p, f, r, e] = w8[idx_r, 2*(f*128+p)+e]; the
matmuls use stride-2 lhsT views with an x layout reordered to match
(x/128 folded in, so no epilogue scale). Outputs accumulate in PSUM
[rows, B] fp32, are copied to SBUF, and leave via two prepared SWDGE
scatters (desc-gen prebuilt during the gather phase; triggers fire after
the copies) to cut the tail HWDGE latency. The host transposes/
assembles per-core outputs and inverse-maps duplicates back to the
original 4403-index order. The older fp16 path (_build_hi) is kept as a
fallback for index distributions that don't fit 512 rows/core.
"""
import os
import sys
from contextlib import ExitStack

sys.path.insert(0, "/opt/trn_rl_repo")
os.environ.setdefault("MYCRO_LOCAL_CACHE", "1")

import numpy as np

D_FF = 11008
D_MODEL = 4096
R_TOTAL = 4403
B = 32
NCORES = 8
P = 128
KT = D_MODEL // P          # 32 contraction tiles (fp16 path)
KT2 = D_MODEL // 256       # 16 u16-granularity tiles (fp8 path)
NV8 = 512                  # padded per-core row count for the fp8 path
W_SCALE = 128.0            # host weight prescale for e3m4 range

_cache = {}


def _build_hi8(gbufs=4):
    """fp8(e3m4) weight-stationary kernel, 512 rows/core in 4x128 chunks."""
    key = ("hi8", gbufs)
    if key in _cache:
        return _cache[key]
    from concourse import bacc, mybir, tile

    f32 = mybir.dt.float32
    f16 = mybir.dt.float16
    f8 = mybir.dt.float8e3
    i16 = mybir.dt.int16

    nch = NV8 // 128  # 4 chunks
    nv16 = NV8 // 16  # 32 idx cols

    nc = bacc.Bacc(
        "TRN2",
        target_bir_lowering=False,
        debug=False,
        enable_asserts=False,
        num_swdge_queues=4,
    )
    whi_dram = nc.dram_tensor("whi", [D_FF, D_MODEL], f8, kind="ExternalInput").ap()
    xq_dram = nc.dram_tensor("xq", [P, KT2 * 2 * B], f16, kind="ExternalInput").ap()
    # first nv16 cols: gather indices; next nv16: iota rows for the scatter
    idx_dram = nc.dram_tensor("idx", [P, 2 * nv16], i16, kind="ExternalInput").ap()
    # 64-wide rows so the scatter elem is 256B (cols 32..63 are zero pad)
    out_dram = nc.dram_tensor("out", [NV8, 2 * B], f32, kind="ExternalOutput").ap()

    # idx DMA issued BEFORE the TileContext entry barrier (~640ns earlier):
    # raw SBUF tensor + manual completion sem; every SWDGE desc-gen that
    # reads it carries an explicit wait
    idx_sb_h = nc.alloc_sbuf_tensor("idx_sb8", [P, 2 * nv16], i16)
    idx_sem = nc.alloc_semaphore("idx_dma")
    nc.gpsimd.sem_clear(idx_sem)
    # NOTE: the wrapped idx layout must cover all 128 partitions -- each of
    # the 8 GPSIMD Q7 cores reads its own 16-partition replica on real HW
    nc.sync.dma_start(idx_sb_h.ap(), idx_dram).then_inc(idx_sem, 16)
    idx_sb = idx_sb_h.ap()

    with tile.TileContext(nc) as tc, ExitStack() as ctx:
        consts = ctx.enter_context(tc.tile_pool(name="consts", bufs=1))
        whi_pool = ctx.enter_context(tc.tile_pool(name="whiT", bufs=gbufs))
        psum = ctx.enter_context(tc.tile_pool(name="psum", bufs=4, space="PSUM"))

        xq_sb = consts.tile([P, KT2 * 2 * B], f16)
        nc.sync.dma_start(xq_sb[:], xq_dram)
        obs = consts.tile([P, nch, 2 * B], f32)
        nc.vector.memset(obs[:], 0)

        # all gather desc-gens first so the Pool engine pipelines ahead of
        # the serialized DMA transfers. The FIRST gather goes through
        # prepare_only + trigger: a triggered transfer skips the 650ns
        # DGE->DMA handoff delay, so the transfer block starts earlier.
        # chunks 0 and 1 via prepare_only + trigger: a triggered transfer
        # skips the 650ns DGE->DMA handoff, so g0 fires right after its
        # desc-gen and g1 packs immediately behind it
        gsems = [nc.alloc_semaphore(f"g{c}_dma") for c in range(2)]
        for s in gsems:
            nc.gpsimd.sem_clear(s)
        wts = []
        for c in range(nch):
            wt = whi_pool.tile([P, KT2 * P * 2], f8, tag=f"wt{c}")
            idx_c = idx_sb[:, c * 8 : (c + 1) * 8]
            if c < 2:
                nc.gpsimd.dma_gather(
                    out_ap=wt[:].rearrange("p (a r) -> p a r", a=KT2 * 2, r=P),
                    in_ap=whi_dram,
                    idxs_ap=idx_c,
                    num_idxs=P,
                    num_idxs_reg=P,
                    elem_size=D_MODEL,
                    transpose=True,
                    prepare_only=True,
                    sem=gsems[c],
                    queue_num=3,
                ).wait_op(idx_sem, 0, "sem-ge")
                if c == 0:
                    # fires g0 only (g1's prep not yet recorded)
                    nc.gpsimd.trigger_dma(count=None, queue_num=3)
            elif c == nch - 1:
                # last chunk in a 10/6 column split: the final exposed
                # transfer is small and its desc-gen still fits the chain
                pieces = [(0, 10), (10, 6)]
                for f0, nf in pieces:
                    nc.gpsimd.dma_gather(
                        out_ap=wt[:].rearrange(
                            "p (a r) -> p a r", a=KT2 * 2, r=P
                        )[:, f0 * 2 : (f0 + nf) * 2, :],
                        in_ap=whi_dram[:, f0 * 256 : (f0 + nf) * 256],
                        idxs_ap=idx_c,
                        num_idxs=P,
                        num_idxs_reg=P,
                        elem_size=nf * 256,
                        elem_step=D_MODEL,
                        transpose=True,
                    ).wait_op(idx_sem, 0, "sem-ge")
            else:
                nc.gpsimd.dma_gather(
                    out_ap=wt[:].rearrange("p (a r) -> p a r", a=KT2 * 2, r=P),
                    in_ap=whi_dram,
                    idxs_ap=idx_c,
                    num_idxs=P,
                    num_idxs_reg=P,
                    elem_size=D_MODEL,
                    transpose=True,
                ).wait_op(idx_sem, 0, "sem-ge")
            wts.append(wt)

        # prepared output scatters: desc-gen runs now (Pool is idle once the
        # gather desc-gens finish); triggers fire after the PSUM copies land.
        # Tile's deferred-dep machinery doesn't bind producers emitted after
        # the prep, so the copy->trigger ordering is a manual sem protocol.
        semA = nc.alloc_semaphore("outA_dma")
        semB = nc.alloc_semaphore("outB_dma")
        cp_sem = nc.alloc_semaphore("cp_done")  # placeholder, rewritten below
        for s in (semA, semB):
            nc.gpsimd.sem_clear(s)
        # disjoint out_dram slices so Tile sees no WAW between the two
        # scatters (a full-tensor out_ap serializes trigger B behind
        # scatter A's DMA completion); B's iota values are slice-relative
        prepA = nc.gpsimd.dma_scatter_add(
            out_ap=out_dram[: (nch - 1) * P, :],
            in_ap=obs[:, : nch - 1, :],
            idxs_ap=idx_sb[:, nv16 : nv16 + (nch - 1) * 8],
            num_idxs=(nch - 1) * P,
            num_idxs_reg=(nch - 1) * P,
            elem_size=2 * B,
            prepare_only=True,
            sem=semA,
            queue_num=1,
        ).wait_op(idx_sem, 0, "sem-ge")
        prepB = nc.gpsimd.dma_scatter_add(
            out_ap=out_dram[(nch - 1) * P :, :],
            in_ap=obs[:, nch - 1 :, :],
            idxs_ap=idx_sb[:, nv16 + (nch - 1) * 8 : nv16 + (nch - 1) * 8 + 5],
            num_idxs=80,
            num_idxs_reg=80,
            elem_size=2 * B,
            prepare_only=True,
            sem=semB,
            queue_num=2,
        ).wait_op(idx_sem, 0, "sem-ge")
        nc.gpsimd.trigger_dma(count=None, queue_num=3)  # fires g1

        xq4 = xq_sb[:].rearrange("p (f e b) -> p f e b", f=KT2, e=2, b=B)
        copy_names = []
        trig_specs = []  # (trigger ins name, gating copy index)
        for c in range(nch):
            w4 = wts[c][:].rearrange("p (f i e) -> p f i e", f=KT2, i=P, e=2)
            psT = psum.tile([P, B], f32, tag="psT8")
            for k in range(2 * KT2):
                f, e = k // 2, k % 2
                nc.tensor.matmul(
                    out=psT[:],
                    lhsT=w4[:, f, :, e],
                    rhs=xq4[:, f, e, :],
                    start=(k == 0),
                    stop=(k == 2 * KT2 - 1),
                )
            if c == nch - 1:
                # DVE: faster PSUM access than Act, and idle this late
                cp = nc.vector.tensor_copy(obs[:, c, :B], psT[:])
            else:
                cp = nc.scalar.copy(obs[:, c, :B], psT[:])
            copy_names.append(cp.ins.name)
            if c == nch - 2:
                tr = nc.gpsimd.trigger_dma(count=None, queue_num=1).wait_op(
                    cp_sem, 0, "sem-ge"
                )
                # ordering-only edge: keep prepB's desc-gen (Pool engine)
                # scheduled before this trigger so it is off the tail
                from concourse.bass import InstructionNameOrderedSet

                _dep = InstructionNameOrderedSet()
                _dep.add(prepB.ins.name)
                tr.ins.add_nosync_dependencies_from(_dep)
                trig_specs.append((tr.ins.name, c))
        tr = nc.gpsimd.trigger_dma(count=None, queue_num=2).wait_op(
            cp_sem, 0, "sem-ge"
        )
        trig_specs.append((tr.ins.name, nch - 1))

    # Post-schedule patches. (a) Tile's deferred-dep machinery doesn't bind
    # obs-copy producers emitted after the preps, so the triggers carry
    # placeholder waits: re-point them at the gating copy's engine-tick sem
    # (value = cumulative tick at that copy), which fires at Act ENGINE
    # completion. (b) The Tile pass assigns the preps DMASW lanes and emits
    # teardown waits on them, but a gen_mode==1 prep's transfer bumps only the
    # descriptor-baked user sem -- the lane sems are never incremented
    # (framework gap). Re-point those orphaned waits at the user DMA sems.
    produced = set()
    all_waits = []
    insts = []
    for blk in nc.m.functions[0].blocks:
        for ins in blk.instructions:
            insts.append(ins)
            si = ins.sync_info
            if not si:
                continue
            for u in si.on_update or []:
                produced.add(u.id)
            for w in si.on_wait or []:
                all_waits.append((ins, w))

    # (a0) idx waits: Tile's internal sim can't see the pre-context idx DMA,
    # so the waits were emitted trivially satisfiable; restore the real value
    for ins, w in all_waits:
        if w.ant_name == "idx_dma" and w.wait_value == 0:
            w.wait_value = 16
            si = ins.sync_info
            si.on_wait = si.on_wait

    # (a) trigger gating: find each copy's engine-tick sem + cumulative value
    by_name = {ins.name: ins for ins in insts}
    copy_sem = {}
    for cn in copy_names:
        cp_ins = by_name[cn]
        ups = [
            u
            for u in (cp_ins.sync_info.on_update or [])
            if u.update_mode == "sem-inc"
        ]
        assert len(ups) == 1, f"copy {cn} tick updates: {ups}"
        copy_sem[cn] = (ups[0].id, ups[0].ant_name)
    ticks = {}
    copy_tick = {}
    for ins in insts:
        si = ins.sync_info
        if si:
            for u in si.on_update or []:
                if u.update_mode == "sem-inc":
                    ticks[u.id] = ticks.get(u.id, 0) + u.update_value
        if ins.name in copy_names:
            copy_tick[ins.name] = ticks.get(copy_sem[ins.name][0], 0)
    for trig_name, c in trig_specs:
        trig = by_name[trig_name]
        si = trig.sync_info
        wl = si.on_wait
        patched = 0
        for w in wl:
            if w.ant_name == "cp_done":
                cn = copy_names[c]
                w.id = copy_sem[cn][0]
                w.wait_value = copy_tick[cn]
                try:
                    w.ant_name = copy_sem[cn][1]
                except Exception:
                    pass
                patched += 1
        assert patched == 1, f"trigger {trig_name}: {patched} placeholder waits"
        si.on_wait = wl
    # the scheduler also emits standalone EventSemaphore pre-waits that
    # duplicate the trigger's own (patched) wait; each costs SEM_DELAY on
    # the tail -- neutralize any wait still naming the placeholder sem
    for ins, w in all_waits:
        if w.ant_name == "cp_done":
            si = ins.sync_info
            wl = si.on_wait
            for w2 in wl:
                if w2.ant_name == "cp_done":
                    w2.wait_value = 0
            si.on_wait = wl
    # emulate pass-1's round-robin DMASW lane assignment to map each
    # gen_mode==1 prep to its (orphaned) lane sem, then re-point every wait
    # on that lane at the prep's user DMA sem
    swdge_types = ("InstDMAGatherAnt", "InstDMAScatterAddAnt")
    lane, n_sw = 0, 0
    lane_to_user = {}
    for ins in insts:
        if (
            str(ins.engine) == "EngineType.Pool"
            and type(ins).__name__ in swdge_types
        ):
            this_lane, lane, n_sw = lane, (lane + 1) % 8, n_sw + 1
            if getattr(ins, "gen_mode", 0) == 1:
                u0 = (ins.sync_info.on_update or [None])[0]
                assert u0 is not None and u0.update_value == 16, str(ins)
                lane_to_user[f"DMASW{this_lane}_"] = (u0.id, u0.ant_name)
    assert n_sw <= 8, f"{n_sw} SWDGE DMAs wrap the 8 DMASW lanes"
    for ins, w in all_waits:
        nm = w.ant_name or ""
        hit = [v for pref, v in lane_to_user.items() if nm.startswith(pref)]
        if hit:
            assert w.wait_value == 16, f"{ins.name}: {w}"
            si = ins.sync_info
            wl = si.on_wait
            for w2 in wl:
                if w2.ant_name == nm:
                    w2.id = hit[0][0]
                    try:
                        w2.ant_name = hit[0][1]
                    except Exception:
                        pass
            si.on_wait = wl

    nc.compile()
    _cache[key] = nc
    return nc


def _wrap_idx(idx_pad):
    """[npad] int16 -> [128, npad//16] wrapped-16 layout, replicated 8x."""
    npad = idx_pad.shape[0]
    blk = idx_pad.reshape(npad // 16, 16).T  # [16, npad//16]
    return np.ascontiguousarray(np.tile(blk, (8, 1)))


def _make_in_maps_hi8(x, weight, indices):
    """Host prep for the fp8 kernel: dedup+shard indices, e3m4 cast.

    Returns (in_maps, assemble_fn) or None if the shape doesn't fit."""
    import ml_dtypes

    x = np.asarray(x, dtype=np.float32)
    weight = np.asarray(weight, dtype=np.float32)
    indices = np.asarray(indices, dtype=np.int64)

    uniq, inv = np.unique(indices, return_inverse=True)
    nu = len(uniq)
    base, rem = divmod(nu, NCORES)
    counts = [base + (1 if c < rem else 0) for c in range(NCORES)]
    # scatter B statically covers rows 384..463 only
    if max(counts) > NV8 - 128 + 80:
        return None
    starts = np.concatenate([[0], np.cumsum(counts)[:-1]])

    whi = np.ascontiguousarray(
        np.clip(weight * W_SCALE, -15.5, 15.5).astype(ml_dtypes.float8_e3m4)
    )
    # xq[p, f, e, b] = fp16(x[b, 0, 2*(f*128+p)+e] / W_SCALE)
    xt = x[:, 0, :].T / W_SCALE               # [4096, B]
    xq = np.ascontiguousarray(
        xt.reshape(KT2, P, 2, B).transpose(1, 0, 2, 3).reshape(P, KT2 * 2 * B)
    ).astype(np.float16)

    # scatter iota: rows 0..383 for scatter A, then slice-relative 0..127
    # for scatter B (its out_ap starts at row 384)
    iota = _wrap_idx(
        np.concatenate(
            [np.arange(NV8 - 128, dtype=np.int16), np.arange(128, dtype=np.int16)]
        )
    )
    in_maps = []
    for c in range(NCORES):
        idx_pad = np.full(NV8, uniq[starts[c]], dtype=np.int16)
        idx_pad[: counts[c]] = uniq[starts[c] : starts[c] + counts[c]]
        idx_full = np.concatenate([_wrap_idx(idx_pad), iota], axis=1)
        in_maps.append(
            {"whi": whi, "xq": xq, "idx": np.ascontiguousarray(idx_full)}
        )

    def assemble(results):
        cols = np.empty((B, nu), dtype=np.float32)
        for c in range(NCORES):
            cols[:, starts[c] : starts[c] + counts[c]] = (
                results[c]["out"][: counts[c], :B].T
            )
        return np.ascontiguousarray(cols[:, inv].reshape(B, 1, R_TOTAL))

    return in_maps, assemble


def _build_hi(nvalid, gbufs=3, reps=1):
    """fp16-only weight-stationary kernel (fallback path)."""
    key = ("hi5", nvalid, gbufs, reps)
    if key in _cache:
        return _cache[key]
    from concourse import bacc, mybir, tile

    f32 = mybir.dt.float32
    f16 = mybir.dt.float16
    i16 = mybir.dt.int16

    nfull = nvalid // 128      # full 128-row transpose-gather chunks
    nt = nvalid % 128          # odd-size tail chunk, non-transpose gather
    nch = nfull + (1 if nt else 0)
    nv16 = nvalid // 16

    nc = bacc.Bacc(
        "TRN2",
        target_bir_lowering=False,
        debug=False,
        enable_asserts=False,
        num_swdge_queues=2,
    )
    whi_dram = nc.dram_tensor("whi", [D_FF, D_MODEL], f16, kind="ExternalInput").ap()
    xh_dram = nc.dram_tensor("xh", [P, KT * B], f16, kind="ExternalInput").ap()
    if nt:
        id_dram = nc.dram_tensor("ident", [nt, nt], f16, kind="ExternalInput").ap()
    idx_dram = nc.dram_tensor("idx", [P, 2 * nv16], i16, kind="ExternalInput").ap()
    out_dram = nc.dram_tensor("out", [nvalid, 2 * B], f32, kind="ExternalOutput").ap()

    with tile.TileContext(nc) as tc, ExitStack() as ctx:
        consts = ctx.enter_context(tc.tile_pool(name="consts", bufs=1))
        whi_pool = ctx.enter_context(tc.tile_pool(name="whiT", bufs=max(gbufs, nch)))
        psum = ctx.enter_context(tc.tile_pool(name="psum", bufs=4, space="PSUM"))

        idx_sb = consts.tile([P, 2 * nv16], i16)
        nc.sync.dma_start(idx_sb[:], idx_dram)
        xh_sb = consts.tile([P, KT * B], f16)
        nc.sync.dma_start(xh_sb[:], xh_dram)
        obs = consts.tile([P, nch, 2 * B], f32)
        nc.vector.memset(obs[:], 0)
        if nt:
            id_sb = consts.tile([nt, nt], f16)
            nc.sync.dma_start(id_sb[:], id_dram)
            w4 = consts.tile([P, 1, D_MODEL], f16)

        whiTs = []
        if nt:
            nc.gpsimd.dma_gather(
                out_ap=w4[:],
                in_ap=whi_dram,
                idxs_ap=idx_sb[:, nfull * 8 : nv16],
                num_idxs=nt,
                num_idxs_reg=nt,
                elem_size=D_MODEL,
                transpose=False,
            )
        for c in range(nfull):
            r0 = c * 128
            whiT = whi_pool.tile([P, KT, 128], f16, tag=f"whiT{c}")
            if c == nfull - 1:
                for h in range(4):
                    nc.gpsimd.dma_gather(
                        out_ap=whiT[:, h * 8 : (h + 1) * 8, :],
                        in_ap=whi_dram[:, h * 1024 : (h + 1) * 1024],
                        idxs_ap=idx_sb[:, r0 // 16 : r0 // 16 + 8],
                        num_idxs=128,
                        num_idxs_reg=128,
                        elem_size=1024,
                        elem_step=D_MODEL,
                        transpose=True,
                    )
            else:
                nc.gpsimd.dma_gather(
                    out_ap=whiT[:],
                    in_ap=whi_dram,
                    idxs_ap=idx_sb[:, r0 // 16 : r0 // 16 + 8],
                    num_idxs=128,
                    num_idxs_reg=128,
                    elem_size=D_MODEL,
                    transpose=True,
                )
            whiTs.append(whiT)
        if nt:
            whiT_nt = whi_pool.tile([P, KT, nt], f16, tag="whiTnt")
            for k in range(KT):
                psX = psum.tile([P, nt], f16, tag="psX")
                nc.tensor.transpose(
                    psX[:], w4[:nt, 0, k * P : (k + 1) * P], id_sb[:]
                )
                nc.vector.tensor_copy(whiT_nt[:, k, :], psX[:])
            whiTs.append(whiT_nt)

        order = list(range(nch))
        if nt and nch >= 2:
            order = order[: nch - 2] + [nch - 1, nch - 2]
        for c in order:
            whiT = whiTs[c]
            rows = nt if (nt and c == nch - 1) else 128
            r0 = nfull * 128 if (nt and c == nch - 1) else c * 128
            psT = psum.tile([rows, B], f32, tag="psT")
            for k in range(KT):
                nc.tensor.matmul(
                    out=psT[:],
                    lhsT=whiT[:, k, :],
                    rhs=xh_sb[:, k * B : (k + 1) * B],
                    start=(k == 0),
                    stop=(k == KT - 1),
                )
            nc.scalar.copy(obs[:rows, c, :B], psT[:])
            nc.sync.dma_start(out_dram[r0 : r0 + rows, :B], obs[:rows, c, :B])

    nc.compile()
    _cache[key] = nc
    return nc


def _make_in_maps_hi(x, weight, indices):
    """Host prep for the fp16 fallback kernel."""
    x = np.asarray(x, dtype=np.float32)
    weight = np.asarray(weight, dtype=np.float32)
    indices = np.asarray(indices, dtype=np.int64)

    whi = np.ascontiguousarray(weight.astype(np.float16))
    xt = np.ascontiguousarray(
        x[:, 0, :].reshape(B, KT, P).transpose(2, 1, 0).reshape(P, KT * B)
    )
    xh = np.ascontiguousarray(xt.astype(np.float16))

    uniq, inv = np.unique(indices, return_inverse=True)
    nu = len(uniq)
    base, rem = divmod(nu, NCORES)
    counts = [base + (1 if c < rem else 0) for c in range(NCORES)]
    starts = np.concatenate([[0], np.cumsum(counts)[:-1]])
    nvalid = -(-max(counts) // 16) * 16

    iota = _wrap_idx(np.arange(nvalid, dtype=np.int16))
    ident = np.eye(nvalid % 128 or 1, dtype=np.float16)
    in_maps = []
    for c in range(NCORES):
        idx_pad = np.zeros(nvalid, dtype=np.int16)
        idx_pad[: counts[c]] = uniq[starts[c] : starts[c] + counts[c]]
        idx_full = np.concatenate([_wrap_idx(idx_pad), iota], axis=1)
        in_maps.append(
            {"whi": whi, "xh": xh, "idx": np.ascontiguousarray(idx_full),
             "ident": ident}
        )

    def assemble(results):
        cols = np.empty((B, nu), dtype=np.float32)
        for c in range(NCORES):
            cols[:, starts[c] : starts[c] + counts[c]] = (
                results[c]["out"][: counts[c], :B].T
            )
        return np.ascontiguousarray(cols[:, inv].reshape(B, 1, R_TOTAL))

    return in_maps, assemble, nvalid


def _filter_in_maps(nc, in_maps):
    names = set()
    from concourse import mybir

    for alloc in nc.m.functions[0].allocations:
        if isinstance(alloc, mybir.MemoryLocationSet) and alloc.kind == "ExternalInput":
            names.add(alloc.memorylocations[0].name)
    return [{k: v for k, v in m.items() if k in names} for m in in_maps]


def run_full(x, weight, indices, trace=False, mode="hi8"):
    """Returns (output, BassKernelResults)."""
    from concourse.bass_utils import run_bass_kernel_spmd

    if mode == "hi8":
        prep = _make_in_maps_hi8(x, weight, indices)
        if prep is not None:
            in_maps, assemble = prep
            nc = _build_hi8()
            in_maps = _filter_in_maps(nc, in_maps)
            res = run_bass_kernel_spmd(nc, in_maps, list(range(NCORES)), trace=trace)
            return assemble(res.results), res
        mode = "hi"

    in_maps, assemble, nvalid = _make_in_maps_hi(x, weight, indices)
    nc = _build_hi(nvalid)
    in_maps = _filter_in_maps(nc, in_maps)
    res = run_bass_kernel_spmd(nc, in_maps, list(range(NCORES)), trace=trace)
    return assemble(res.results), res


def kernel(x, weight, indices):
    out, _ = run_full(x, weight, indices)
    return out


# revision 43
# speedup vs baseline: 1.5031x; 1.0015x over previous
"""Trainium2 Bass kernel for gathered-row MLP decode matmul.

out[b, 0, r] = sum_d x[b, 0, d] * weight[indices[r], d]

Active path (kernel() -> run_full(mode="hi8") -> _build_hi8): dedup the
indices on the host, shard the ~3629 unique rows across 8 cores (~454
each, padded to 512 = 4x128 with duplicate indices). The weight is cast
to fp8 e3m4 on the host with a x128 prescale (4-bit mantissa; end-to-end
scale-rel error ~1.2e-2 against the 2e-2 gate), halving HBM gather
traffic vs fp16. The transpose-gather DMA works at 16-bit granularity,
so fp8 values land as interleaved even/odd pairs
[p, f, r, e] = w8[idx_r, 2*(f*128+p)+e]; the matmuls use stride-2 lhsT
views with an x layout reordered to match (x/128 folded in, so no
epilogue scale). Critical-path engineering, all verified against the
TimelineSim cost model and real-HW runs:
  - the gather-index DMA is issued before the TileContext entry barrier
    (raw SBUF tensor + manual sem), and split from the scatter-iota half
    so desc-gen starts as early as possible;
  - the first two 128-row gathers go through prepare_only + trigger_dma,
    skipping the 650ns DGE->DMA handoff so the five gather transfers run
    back-to-back on the DMA engines from ~4.0us;
  - the last chunk is fetched as 10/6 column pieces so only a 546ns
    transfer plus 12 matmuls are exposed on the tail;
  - outputs accumulate in PSUM [rows, B] fp32, are copied to SBUF (last
    chunk via the faster DVE path), and leave via two prepared SWDGE
    scatters with disjoint out_dram slices (avoids a false WAW edge),
    triggered right after the copies land.
Tile's deferred-dep machinery does not bind producers emitted after a
prep and never increments the DMASW lane sems of gen_mode==1 preps, so
_build_hi8 post-processes the scheduled BIR: trigger placeholder waits
are re-pointed at the gating copy's engine-tick sem, and orphaned DMASW
lane waits at the preps' user DMA sems. The host transposes/assembles
per-core outputs and inverse-maps duplicates back to the original
4403-index order. The older fp16 path (_build_hi) is kept as a fallback
for index distributions that don't fit 464 rows/core.
"""
import os
import sys
from contextlib import ExitStack

sys.path.insert(0, "/opt/trn_rl_repo")
os.environ.setdefault("MYCRO_LOCAL_CACHE", "1")

import numpy as np

D_FF = 11008
D_MODEL = 4096
R_TOTAL = 4403
B = 32
NCORES = 8
P = 128
KT = D_MODEL // P          # 32 contraction tiles (fp16 path)
KT2 = D_MODEL // 256       # 16 u16-granularity tiles (fp8 path)
NV8 = 512                  # padded per-core row count for the fp8 path
W_SCALE = 128.0            # host weight prescale for e3m4 range

_cache = {}
_FILLS = (0, 0, 0, 0)


def _build_hi8(gbufs=4):
    """fp8(e3m4) weight-stationary kernel, 512 rows/core in 4x128 chunks."""
    key = ("hi8", gbufs)
    if key in _cache:
        return _cache[key]
    from concourse import bacc, mybir, tile

    f32 = mybir.dt.float32
    f16 = mybir.dt.float16
    f8 = mybir.dt.float8e3
    i16 = mybir.dt.int16

    nch = NV8 // 128  # 4 chunks
    nv16 = NV8 // 16  # 32 idx cols

    nc = bacc.Bacc(
        "TRN2",
        target_bir_lowering=False,
        debug=False,
        enable_asserts=False,
        num_swdge_queues=4,
    )
    whi_dram = nc.dram_tensor("whi", [D_FF, D_MODEL], f8, kind="ExternalInput").ap()
    xq_dram = nc.dram_tensor("xq", [P, KT2 * 2 * B], f16, kind="ExternalInput").ap()
    # first nv16 cols: gather indices; next nv16: iota rows for the scatter
    idx_dram = nc.dram_tensor("idx", [P, 2 * nv16], i16, kind="ExternalInput").ap()
    # 64-wide rows so the scatter elem is 256B (cols 32..63 are zero pad)
    out_dram = nc.dram_tensor("out", [NV8, 2 * B], f32, kind="ExternalOutput").ap()

    # idx DMA issued BEFORE the TileContext entry barrier (~640ns earlier):
    # raw SBUF tensor + manual completion sem; every SWDGE desc-gen that
    # reads it carries an explicit wait
    idx_sb_h = nc.alloc_sbuf_tensor("idx_sb8", [P, 2 * nv16], i16)
    idx_sem = nc.alloc_semaphore("idx_dma")
    nc.gpsimd.sem_clear(idx_sem)
    # NOTE: the wrapped idx layout must cover all 128 partitions -- each of
    # the 8 GPSIMD Q7 cores reads its own 16-partition replica on real HW.
    # Gather-index half first (smaller transfer -> earlier desc-gen); the
    # scatter-iota half follows on its own sem (preps run much later).
    idx2_sem = nc.alloc_semaphore("idx2_dma")
    nc.gpsimd.sem_clear(idx2_sem)
    nc.sync.dma_start(
        idx_sb_h.ap()[:, :nv16], idx_dram[:, :nv16]
    ).then_inc(idx_sem, 16)
    nc.sync.dma_start(
        idx_sb_h.ap()[:, nv16:], idx_dram[:, nv16:]
    ).then_inc(idx2_sem, 16)
    idx_sb = idx_sb_h.ap()

    with tile.TileContext(nc) as tc, ExitStack() as ctx:
        consts = ctx.enter_context(tc.tile_pool(name="consts", bufs=1))
        whi_pool = ctx.enter_context(tc.tile_pool(name="whiT", bufs=gbufs))
        psum = ctx.enter_context(tc.tile_pool(name="psum", bufs=4, space="PSUM"))

        xq_sb = consts.tile([P, KT2 * 2 * B], f16)
        nc.sync.dma_start(xq_sb[:], xq_dram)
        obs = consts.tile([P, nch, 2 * B], f32)
        nc.vector.memset(obs[:], 0)

        # all gather desc-gens first so the Pool engine pipelines ahead of
        # the serialized DMA transfers. The FIRST gather goes through
        # prepare_only + trigger: a triggered transfer skips the 650ns
        # DGE->DMA handoff delay, so the transfer block starts earlier.
        # chunks 0 and 1 via prepare_only + trigger: a triggered transfer
        # skips the 650ns DGE->DMA handoff, so g0 fires right after its
        # desc-gen and g1 packs immediately behind it
        gsems = [nc.alloc_semaphore(f"g{c}_dma") for c in range(2)]
        for s in gsems:
            nc.gpsimd.sem_clear(s)
        wts = []
        for c in range(nch):
            wt = whi_pool.tile([P, KT2 * P * 2], f8, tag=f"wt{c}")
            idx_c = idx_sb[:, c * 8 : (c + 1) * 8]
            if c < 2:
                nc.gpsimd.dma_gather(
                    out_ap=wt[:].rearrange("p (a r) -> p a r", a=KT2 * 2, r=P),
                    in_ap=whi_dram,
                    idxs_ap=idx_c,
                    num_idxs=P,
                    num_idxs_reg=P,
                    elem_size=D_MODEL,
                    transpose=True,
                    prepare_only=True,
                    sem=gsems[c],
                    queue_num=3,
                ).wait_op(idx_sem, 0, "sem-ge")
                if c == 0:
                    # fires g0 only (g1's prep not yet recorded)
                    nc.gpsimd.trigger_dma(count=None, queue_num=3)
            elif c == nch - 1:
                # last chunk in a 10/6 column split: the final exposed
                # transfer is small and its desc-gen still fits the chain
                pieces = [(0, 10), (10, 6)]
                for f0, nf in pieces:
                    nc.gpsimd.dma_gather(
                        out_ap=wt[:].rearrange(
                            "p (a r) -> p a r", a=KT2 * 2, r=P
                        )[:, f0 * 2 : (f0 + nf) * 2, :],
                        in_ap=whi_dram[:, f0 * 256 : (f0 + nf) * 256],
                        idxs_ap=idx_c,
                        num_idxs=P,
                        num_idxs_reg=P,
                        elem_size=nf * 256,
                        elem_step=D_MODEL,
                        transpose=True,
                    ).wait_op(idx_sem, 0, "sem-ge")
            else:
                nc.gpsimd.dma_gather(
                    out_ap=wt[:].rearrange("p (a r) -> p a r", a=KT2 * 2, r=P),
                    in_ap=whi_dram,
                    idxs_ap=idx_c,
                    num_idxs=P,
                    num_idxs_reg=P,
                    elem_size=D_MODEL,
                    transpose=True,
                ).wait_op(idx_sem, 0, "sem-ge")
            wts.append(wt)

        # prepared output scatters: desc-gen runs now (Pool is idle once the
        # gather desc-gens finish); triggers fire after the PSUM copies land.
        # Tile's deferred-dep machinery doesn't bind producers emitted after
        # the prep, so the copy->trigger ordering is a manual sem protocol.
        semA = nc.alloc_semaphore("outA_dma")
        semB = nc.alloc_semaphore("outB_dma")
        cp_sem = nc.alloc_semaphore("cp_done")  # placeholder, rewritten below
        for s in (semA, semB):
            nc.gpsimd.sem_clear(s)
        # standalone Pool wait: the scatter preps' desc-gen reads the iota
        # half of idx (separate DMA); Pool is in-order so one wait covers both
        nc.gpsimd.wait_ge(idx2_sem, 0)
        # disjoint out_dram slices so Tile sees no WAW between the two
        # scatters (a full-tensor out_ap serializes trigger B behind
        # scatter A's DMA completion); B's iota values are slice-relative
        prepA = nc.gpsimd.dma_scatter_add(
            out_ap=out_dram[: (nch - 1) * P, :],
            in_ap=obs[:, : nch - 1, :],
            idxs_ap=idx_sb[:, nv16 : nv16 + (nch - 1) * 8],
            num_idxs=(nch - 1) * P,
            num_idxs_reg=(nch - 1) * P,
            elem_size=2 * B,
            prepare_only=True,
            sem=semA,
            queue_num=1,
        ).wait_op(idx_sem, 0, "sem-ge")
        prepB = nc.gpsimd.dma_scatter_add(
            out_ap=out_dram[(nch - 1) * P :, :],
            in_ap=obs[:, nch - 1 :, :],
            idxs_ap=idx_sb[:, nv16 + (nch - 1) * 8 : nv16 + (nch - 1) * 8 + 5],
            num_idxs=80,
            num_idxs_reg=80,
            elem_size=2 * B,
            prepare_only=True,
            sem=semB,
            queue_num=2,
        ).wait_op(idx_sem, 0, "sem-ge")
        nc.gpsimd.trigger_dma(count=None, queue_num=3)  # fires g1

        xq4 = xq_sb[:].rearrange("p (f e b) -> p f e b", f=KT2, e=2, b=B)
        fill_pool = ctx.enter_context(
            tc.tile_pool(name="fillps", bufs=1, space="PSUM")
        )
        fill_ps = fill_pool.tile([P, B], f32, name="fill_ps")
        w40 = wts[0][:].rearrange("p (f i e) -> p f i e", f=KT2, i=P, e=2)

        def _fill(n):
            # p-state fillers: keep the PE busy across data-wait gaps so the
            # clock ramp survives to the tail matmuls (chunk0's tile is
            # available data; results discarded)
            for _ in range(n):
                nc.tensor.matmul(
                    out=fill_ps[:],
                    lhsT=w40[:, 0, :, 0],
                    rhs=xq4[:, 0, 0, :],
                    start=True,
                    stop=True,
                )

        copy_names = []
        trig_specs = []  # (trigger ins name, gating copy index)
        fills = _FILLS
        for c in range(nch):
            w4 = wts[c][:].rearrange("p (f i e) -> p f i e", f=KT2, i=P, e=2)
            psT = psum.tile([P, B], f32, tag="psT8")
            for k in range(2 * KT2):
                f, e = k // 2, k % 2
                nc.tensor.matmul(
                    out=psT[:],
                    lhsT=w4[:, f, :, e],
                    rhs=xq4[:, f, e, :],
                    start=(k == 0),
                    stop=(k == 2 * KT2 - 1),
                )
                if c == nch - 1 and k == 19:
                    _fill(fills[3])
            if c == nch - 1:
                # DVE: faster PSUM access than Act, and idle this late
                cp = nc.vector.tensor_copy(obs[:, c, :B], psT[:])
            else:
                cp = nc.scalar.copy(obs[:, c, :B], psT[:])
            copy_names.append(cp.ins.name)
            if c < nch - 1:
                _fill(fills[c])
            if c == nch - 2:
                tr = nc.gpsimd.trigger_dma(count=None, queue_num=1).wait_op(
                    cp_sem, 0, "sem-ge"
                )
                # ordering-only edge: keep prepB's desc-gen (Pool engine)
                # scheduled before this trigger so it is off the tail
                from concourse.bass import InstructionNameOrderedSet

                _dep = InstructionNameOrderedSet()
                _dep.add(prepB.ins.name)
                tr.ins.add_nosync_dependencies_from(_dep)
                trig_specs.append((tr.ins.name, c))
        tr = nc.gpsimd.trigger_dma(count=None, queue_num=2).wait_op(
            cp_sem, 0, "sem-ge"
        )
        trig_specs.append((tr.ins.name, nch - 1))

    # Post-schedule patches. (a) Tile's deferred-dep machinery doesn't bind
    # obs-copy producers emitted after the preps, so the triggers carry
    # placeholder waits: re-point them at the gating copy's engine-tick sem
    # (value = cumulative tick at that copy), which fires at Act ENGINE
    # completion. (b) The Tile pass assigns the preps DMASW lanes and emits
    # teardown waits on them, but a gen_mode==1 prep's transfer bumps only the
    # descriptor-baked user sem -- the lane sems are never incremented
    # (framework gap). Re-point those orphaned waits at the user DMA sems.
    produced = set()
    all_waits = []
    insts = []
    for blk in nc.m.functions[0].blocks:
        for ins in blk.instructions:
            insts.append(ins)
            si = ins.sync_info
            if not si:
                continue
            for u in si.on_update or []:
                produced.add(u.id)
            for w in si.on_wait or []:
                all_waits.append((ins, w))

    # (a0) idx waits: Tile's internal sim can't see the pre-context idx DMA,
    # so the waits were emitted trivially satisfiable; restore the real value
    for ins, w in all_waits:
        if w.ant_name in ("idx_dma", "idx2_dma") and w.wait_value == 0:
            w.wait_value = 16
            si = ins.sync_info
            si.on_wait = si.on_wait

    # (a) trigger gating: find each copy's engine-tick sem + cumulative value
    by_name = {ins.name: ins for ins in insts}
    copy_sem = {}
    for cn in copy_names:
        cp_ins = by_name[cn]
        ups = [
            u
            for u in (cp_ins.sync_info.on_update or [])
            if u.update_mode == "sem-inc"
        ]
        assert len(ups) == 1, f"copy {cn} tick updates: {ups}"
        copy_sem[cn] = (ups[0].id, ups[0].ant_name)
    ticks = {}
    copy_tick = {}
    for ins in insts:
        si = ins.sync_info
        if si:
            for u in si.on_update or []:
                if u.update_mode == "sem-inc":
                    ticks[u.id] = ticks.get(u.id, 0) + u.update_value
        if ins.name in copy_names:
            copy_tick[ins.name] = ticks.get(copy_sem[ins.name][0], 0)
    for trig_name, c in trig_specs:
        trig = by_name[trig_name]
        si = trig.sync_info
        wl = si.on_wait
        patched = 0
        for w in wl:
            if w.ant_name == "cp_done":
                cn = copy_names[c]
                w.id = copy_sem[cn][0]
                w.wait_value = copy_tick[cn]
                try:
                    w.ant_name = copy_sem[cn][1]
                except Exception:
                    pass
                patched += 1
        assert patched == 1, f"trigger {trig_name}: {patched} placeholder waits"
        si.on_wait = wl
    # the scheduler also emits standalone EventSemaphore pre-waits that
    # duplicate the trigger's own (patched) wait; each costs SEM_DELAY on
    # the tail -- neutralize any wait still naming the placeholder sem
    for ins, w in all_waits:
        if w.ant_name == "cp_done":
            si = ins.sync_info
            wl = si.on_wait
            for w2 in wl:
                if w2.ant_name == "cp_done":
                    w2.wait_value = 0
            si.on_wait = wl
    # emulate pass-1's round-robin DMASW lane assignment to map each
    # gen_mode==1 prep to its (orphaned) lane sem, then re-point every wait
    # on that lane at the prep's user DMA sem
    swdge_types = ("InstDMAGatherAnt", "InstDMAScatterAddAnt")
    lane, n_sw = 0, 0
    lane_to_user = {}
    for ins in insts:
        if (
            str(ins.engine) == "EngineType.Pool"
            and type(ins).__name__ in swdge_types
        ):
            this_lane, lane, n_sw = lane, (lane + 1) % 8, n_sw + 1
            if getattr(ins, "gen_mode", 0) == 1:
                u0 = (ins.sync_info.on_update or [None])[0]
                assert u0 is not None and u0.update_value == 16, str(ins)
                lane_to_user[f"DMASW{this_lane}_"] = (u0.id, u0.ant_name)
    assert n_sw <= 8, f"{n_sw} SWDGE DMAs wrap the 8 DMASW lanes"
    for ins, w in all_waits:
        nm = w.ant_name or ""
        hit = [v for pref, v in lane_to_user.items() if nm.startswith(pref)]
        if hit:
            assert w.wait_value == 16, f"{ins.name}: {w}"
            si = ins.sync_info
            wl = si.on_wait
            for w2 in wl:
                if w2.ant_name == nm:
                    w2.id = hit[0][0]
                    try:
                        w2.ant_name = hit[0][1]
                    except Exception:
                        pass
            si.on_wait = wl

    nc.compile()
    _cache[key] = nc
    return nc


def _wrap_idx(idx_pad):
    """[npad] int16 -> [128, npad//16] wrapped-16 layout, replicated 8x."""
    npad = idx_pad.shape[0]
    blk = idx_pad.reshape(npad // 16, 16).T  # [16, npad//16]
    return np.ascontiguousarray(np.tile(blk, (8, 1)))


def _make_in_maps_hi8(x, weight, indices):
    """Host prep for the fp8 kernel: dedup+shard indices, e3m4 cast.

    Returns (in_maps, assemble_fn) or None if the shape doesn't fit."""
    import ml_dtypes

    x = np.asarray(x, dtype=np.float32)
    weight = np.asarray(weight, dtype=np.float32)
    indices = np.asarray(indices, dtype=np.int64)

    uniq, inv = np.unique(indices, return_inverse=True)
    nu = len(uniq)
    base, rem = divmod(nu, NCORES)
    counts = [base + (1 if c < rem else 0) for c in range(NCORES)]
    # scatter B statically covers rows 384..463 only
    if max(counts) > NV8 - 128 + 80:
        return None
    starts = np.concatenate([[0], np.cumsum(counts)[:-1]])

    whi = np.ascontiguousarray(
        np.clip(weight * W_SCALE, -15.5, 15.5).astype(ml_dtypes.float8_e3m4)
    )
    # xq[p, f, e, b] = fp16(x[b, 0, 2*(f*128+p)+e] / W_SCALE)
    xt = x[:, 0, :].T / W_SCALE               # [4096, B]
    xq = np.ascontiguousarray(
        xt.reshape(KT2, P, 2, B).transpose(1, 0, 2, 3).reshape(P, KT2 * 2 * B)
    ).astype(np.float16)

    # scatter iota: rows 0..383 for scatter A, then slice-relative 0..127
    # for scatter B (its out_ap starts at row 384)
    iota = _wrap_idx(
        np.concatenate(
            [np.arange(NV8 - 128, dtype=np.int16), np.arange(128, dtype=np.int16)]
        )
    )
    in_maps = []
    for c in range(NCORES):
        idx_pad = np.full(NV8, uniq[starts[c]], dtype=np.int16)
        idx_pad[: counts[c]] = uniq[starts[c] : starts[c] + counts[c]]
        idx_full = np.concatenate([_wrap_idx(idx_pad), iota], axis=1)
        in_maps.append(
            {"whi": whi, "xq": xq, "idx": np.ascontiguousarray(idx_full)}
        )

    def assemble(results):
        cols = np.empty((B, nu), dtype=np.float32)
        for c in range(NCORES):
            cols[:, starts[c] : starts[c] + counts[c]] = (
                results[c]["out"][: counts[c], :B].T
            )
        return np.ascontiguousarray(cols[:, inv].reshape(B, 1, R_TOTAL))

    return in_maps, assemble


def _build_hi(nvalid, gbufs=3, reps=1):
    """fp16-only weight-stationary kernel (fallback path)."""
    key = ("hi5", nvalid, gbufs, reps)
    if key in _cache:
        return _cache[key]
    from concourse import bacc, mybir, tile

    f32 = mybir.dt.float32
    f16 = mybir.dt.float16
    i16 = mybir.dt.int16

    nfull = nvalid // 128      # full 128-row transpose-gather chunks
    nt = nvalid % 128          # odd-size tail chunk, non-transpose gather
    nch = nfull + (1 if nt else 0)
    nv16 = nvalid // 16

    nc = bacc.Bacc(
        "TRN2",
        target_bir_lowering=False,
        debug=False,
        enable_asserts=False,
        num_swdge_queues=2,
    )
    whi_dram = nc.dram_tensor("whi", [D_FF, D_MODEL], f16, kind="ExternalInput").ap()
    xh_dram = nc.dram_tensor("xh", [P, KT * B], f16, kind="ExternalInput").ap()
    if nt:
        id_dram = nc.dram_tensor("ident", [nt, nt], f16, kind="ExternalInput").ap()
    idx_dram = nc.dram_tensor("idx", [P, 2 * nv16], i16, kind="ExternalInput").ap()
    out_dram = nc.dram_tensor("out", [nvalid, 2 * B], f32, kind="ExternalOutput").ap()

    with tile.TileContext(nc) as tc, ExitStack() as ctx:
        consts = ctx.enter_context(tc.tile_pool(name="consts", bufs=1))
        whi_pool = ctx.enter_context(tc.tile_pool(name="whiT", bufs=max(gbufs, nch)))
        psum = ctx.enter_context(tc.tile_pool(name="psum", bufs=4, space="PSUM"))

        idx_sb = consts.tile([P, 2 * nv16], i16)
        nc.sync.dma_start(idx_sb[:], idx_dram)
        xh_sb = consts.tile([P, KT * B], f16)
        nc.sync.dma_start(xh_sb[:], xh_dram)
        obs = consts.tile([P, nch, 2 * B], f32)
        nc.vector.memset(obs[:], 0)
        if nt:
            id_sb = consts.tile([nt, nt], f16)
            nc.sync.dma_start(id_sb[:], id_dram)
            w4 = consts.tile([P, 1, D_MODEL], f16)

        whiTs = []
        if nt:
            nc.gpsimd.dma_gather(
                out_ap=w4[:],
                in_ap=whi_dram,
                idxs_ap=idx_sb[:, nfull * 8 : nv16],
                num_idxs=nt,
                num_idxs_reg=nt,
                elem_size=D_MODEL,
                transpose=False,
            )
        for c in range(nfull):
            r0 = c * 128
            whiT = whi_pool.tile([P, KT, 128], f16, tag=f"whiT{c}")
            if c == nfull - 1:
                for h in range(4):
                    nc.gpsimd.dma_gather(
                        out_ap=whiT[:, h * 8 : (h + 1) * 8, :],
                        in_ap=whi_dram[:, h * 1024 : (h + 1) * 1024],
                        idxs_ap=idx_sb[:, r0 // 16 : r0 // 16 + 8],
                        num_idxs=128,
                        num_idxs_reg=128,
                        elem_size=1024,
                        elem_step=D_MODEL,
                        transpose=True,
                    )
            else:
                nc.gpsimd.dma_gather(
                    out_ap=whiT[:],
                    in_ap=whi_dram,
                    idxs_ap=idx_sb[:, r0 // 16 : r0 // 16 + 8],
                    num_idxs=128,
                    num_idxs_reg=128,
                    elem_size=D_MODEL,
                    transpose=True,
                )
            whiTs.append(whiT)
        if nt:
            whiT_nt = whi_pool.tile([P, KT, nt], f16, tag="whiTnt")
            for k in range(KT):
                psX = psum.tile([P, nt], f16, tag="psX")
                nc.tensor.transpose(
                    psX[:], w4[:nt, 0, k * P : (k + 1) * P], id_sb[:]
                )
                nc.vector.tensor_copy(whiT_nt[:, k, :], psX[:])
            whiTs.append(whiT_nt)

        order = list(range(nch))
        if nt and nch >= 2:
            order = order[: nch - 2] + [nch - 1, nch - 2]
        for c in order:
            whiT = whiTs[c]
            rows = nt if (nt and c == nch - 1) else 128
            r0 = nfull * 128 if (nt and c == nch - 1) else c * 128
            psT = psum.tile([rows, B], f32, tag="psT")
            for k in range(KT):
                nc.tensor.matmul(
                    out=psT[:],
                    lhsT=whiT[:, k, :],
                    rhs=xh_sb[:, k * B : (k + 1) * B],
                    start=(k == 0),
                    stop=(k == KT - 1),
                )
            nc.scalar.copy(obs[:rows, c, :B], psT[:])
            nc.sync.dma_start(out_dram[r0 : r0 + rows, :B], obs[:rows, c, :B])

    nc.compile()
    _cache[key] = nc
    return nc


def _make_in_maps_hi(x, weight, indices):
    """Host prep for the fp16 fallback kernel."""
    x = np.asarray(x, dtype=np.float32)
    weight = np.asarray(weight, dtype=np.float32)
    indices = np.asarray(indices, dtype=np.int64)

    whi = np.ascontiguousarray(weight.astype(np.float16))
    xt = np.ascontiguousarray(
        x[:, 0, :].reshape(B, KT, P).transpose(2, 1, 0).reshape(P, KT * B)
    )
    xh = np.ascontiguousarray(xt.astype(np.float16))

    uniq, inv = np.unique(indices, return_inverse=True)
    nu = len(uniq)
    base, rem = divmod(nu, NCORES)
    counts = [base + (1 if c < rem else 0) for c in range(NCORES)]
    starts = np.concatenate([[0], np.cumsum(counts)[:-1]])
    nvalid = -(-max(counts) // 16) * 16

    iota = _wrap_idx(np.arange(nvalid, dtype=np.int16))
    ident = np.eye(nvalid % 128 or 1, dtype=np.float16)
    in_maps = []
    for c in range(NCORES):
        idx_pad = np.zeros(nvalid, dtype=np.int16)
        idx_pad[: counts[c]] = uniq[starts[c] : starts[c] + counts[c]]
        idx_full = np.concatenate([_wrap_idx(idx_pad), iota], axis=1)
        in_maps.append(
            {"whi": whi, "xh": xh, "idx": np.ascontiguousarray(idx_full),
             "ident": ident}
        )

    def assemble(results):
        cols = np.empty((B, nu), dtype=np.float32)
        for c in range(NCORES):
            cols[:, starts[c] : starts[c] + counts[c]] = (
                results[c]["out"][: counts[c], :B].T
            )
        return np.ascontiguousarray(cols[:, inv].reshape(B, 1, R_TOTAL))

    return in_maps, assemble, nvalid


def _filter_in_maps(nc, in_maps):
    names = set()
    from concourse import mybir

    for alloc in nc.m.functions[0].allocations:
        if isinstance(alloc, mybir.MemoryLocationSet) and alloc.kind == "ExternalInput":
            names.add(alloc.memorylocations[0].name)
    return [{k: v for k, v in m.items() if k in names} for m in in_maps]


def run_full(x, weight, indices, trace=False, mode="hi8"):
    """Returns (output, BassKernelResults)."""
    from concourse.bass_utils import run_bass_kernel_spmd

    if mode == "hi8":
        prep = _make_in_maps_hi8(x, weight, indices)
        if prep is not None:
            in_maps, assemble = prep
            nc = _build_hi8()
            in_maps = _filter_in_maps(nc, in_maps)
            res = run_bass_kernel_spmd(nc, in_maps, list(range(NCORES)), trace=trace)
            return assemble(res.results), res
        mode = "hi"

    in_maps, assemble, nvalid = _make_in_maps_hi(x, weight, indices)
    nc = _build_hi(nvalid)
    in_maps = _filter_in_maps(nc, in_maps)
    res = run_bass_kernel_spmd(nc, in_maps, list(range(NCORES)), trace=trace)
    return assemble(res.results), res


def kernel(x, weight, indices):
    out, _ = run_full(x, weight, indices)
    return out
